# revision 10
# baseline (speedup 1.0000x reference)
"""HGNN metapath GRU + edge-softmax message passing on 8 TRN2 NeuronCores.

Strategy (self-contained, full inputs in / full output out):
 - Edges sharded by DESTINATION NODE RANGE: core c owns nodes
   [c*2500, (c+1)*2500) and every edge whose dst lands there (host sorts
   edges by dst).  All segment ops are core-local: zero collectives.
 - GRU h-recurrence matmuls run in fp8(e4m3) DoubleRow perf mode (2 k-chunks
   per instruction, 2x PE throughput).  All GRU weights are pre-scaled by
   S = 128*S_w on the host; activations divide out S via their input scale,
   so the scaling costs zero extra ops.  The h state is kept in bf16
   (accuracy) with an fp8 x128 copy feeding the matmuls.
 - x arrives host-transposed in bf16, so the embedding phase needs no
   input transposes and half the DMA.
 - The final two linear layers are folded through the segment-sum via
   one-hot matmuls (host-precomputed one-hot matrices), as before.
"""

import sys
import numpy as np

sys.path.insert(0, "/opt/trn_rl_repo")

import ml_dtypes  # noqa: E402

N_NODES = 20000
N_CORES = 8
NPC = N_NODES // N_CORES          # 2500 nodes per core
NODE_CHUNKS = (NPC + 127) // 128  # 20
WALK = 4
FEAT = 256
HID = 64
NH = 8
HR = NH * HID                     # 512
G3 = 3 * HR                       # 1536
OUT_DIM = 16
E_TILE = 512
NP_PAD = ((N_NODES + 511) // 512) * 512  # 20480 padded node rows

bf = ml_dtypes.bfloat16
f8np = ml_dtypes.float8_e4m3


def _wrap_idx(v):
    """int array [n] -> wrapped int16 [128, n//16] layout for dma_gather."""
    n = v.shape[0]
    assert n % 16 == 0
    w = v.reshape(n // 16, 16).T.astype(np.int16)      # [16, n//16]
    return np.tile(w, (8, 1))                           # [128, n//16]


def _host_prep(x, W_mlp, b_mlp, W_ih, W_hh, b_ih, b_hh, attn, W_emb, b_emb,
               W_last, b_last, edge_metapath_indices):
    idx = np.asarray(edge_metapath_indices).astype(np.int64)
    dst = idx[:, -1]
    core = np.clip(dst // NPC, 0, N_CORES - 1)

    per_core_eids = []
    for c in range(N_CORES):
        sel = np.nonzero(core == c)[0]
        order = np.argsort(dst[sel], kind="stable")
        per_core_eids.append(sel[order])
    counts = [len(e) for e in per_core_eids]
    E_pad = max(512, ((max(counts) + E_TILE - 1) // E_TILE) * E_TILE)
    n_tiles = E_pad // E_TILE
    n_ech = E_pad // 128

    # per-core sorted/padded indices + local dst
    sidx = np.zeros((N_CORES, E_pad, WALK), np.int64)
    ldst = np.full((N_CORES, E_pad), -1000, np.int64)
    for c in range(N_CORES):
        e = per_core_eids[c]
        sidx[c, :len(e)] = idx[e]
        ldst[c, :len(e)] = dst[e] - c * NPC

    # gather indices: per tile, 4*E_TILE idxs (step-major)
    gidx = np.zeros((N_CORES, n_tiles, 128, (WALK * E_TILE) // 16), np.int16)
    for c in range(N_CORES):
        for t in range(n_tiles):
            v = sidx[c, t * E_TILE:(t + 1) * E_TILE, :].T.reshape(-1)
            gidx[c, t] = _wrap_idx(v)

    # shared scatter schedule: union over cores of node-chunks touched per edge-chunk
    pairs = []
    pair_of = {}
    for k in range(n_ech):
        js = set()
        for c in range(N_CORES):
            d = ldst[c, k * 128:(k + 1) * 128]
            js |= set((d[d >= 0] // 128).tolist())
        if js:
            for j in range(min(js), max(js) + 1):
                pair_of[(k, j)] = len(pairs)
                pairs.append((k, j))
    first_k, last_k = {}, {}
    for (k, j) in pairs:
        first_k.setdefault(j, k)
        last_k[j] = k
    active, max_active = set(), 0
    for k in range(n_ech):
        for (kk, j) in pairs:
            if kk == k and first_k[j] == k:
                active.add(j)
        max_active = max(max_active, len(active))
        for j in list(active):
            if last_k[j] == k:
                active.discard(j)
    n_pairs = len(pairs)

    oneh = np.zeros((N_CORES, max(n_pairs, 1), 128, 128), bf)
    m_ids = np.arange(128)
    for c in range(N_CORES):
        for p, (k, j) in enumerate(pairs):
            d = ldst[c, k * 128:(k + 1) * 128]
            oneh[c, p] = (d[:, None] == (j * 128 + m_ids)[None, :]).astype(bf)

    # ---- weights ----
    W_hh = np.asarray(W_hh, np.float32)
    W_ih = np.asarray(W_ih, np.float32)
    S_w = 2.0 ** np.floor(np.log2(224.0 / max(float(np.abs(W_hh).max()), 1e-30)))
    S = 128.0 * S_w
    inv_S = 1.0 / S

    W_hhT = W_hh.T                                              # [512, 1536]
    whh8 = (W_hhT.reshape(4, 128, G3).transpose(1, 0, 2) * S_w)
    whh8 = np.clip(whh8, -240.0, 240.0).astype(f8np).reshape(128, 4 * G3)
    wihS = np.ascontiguousarray((W_ih.T * S).astype(bf))        # [64, 1536]

    Wc = (np.asarray(W_last, np.float32) @ np.asarray(W_emb, np.float32))
    BA = np.zeros((HR, 136), np.float32)
    attn = np.asarray(attn, np.float32)
    for h in range(NH):
        BA[h * HID:(h + 1) * HID, h * OUT_DIM:(h + 1) * OUT_DIM] = \
            Wc[:, h * HID:(h + 1) * HID].T
        BA[h * HID:(h + 1) * HID, 128 + h] = attn[0, h, :]
    ba_p = BA.reshape(4, 128, 136).transpose(1, 0, 2).reshape(128, 4 * 136).astype(bf)

    b_ih = np.asarray(b_ih, np.float32)
    b_hh = np.asarray(b_hh, np.float32)
    brz = (b_ih + b_hh)[:2 * HR].reshape(8, 128).T.copy()        # [128, 8]
    bnih = b_ih[2 * HR:].reshape(4, 128).T.copy()                # [128, 4]
    bnhhS = (b_hh[2 * HR:] * S).reshape(4, 128).T.copy()         # [128, 4] pre-scaled
    has_bnhh = bool(np.any(b_hh[2 * HR:] != 0.0))

    b_mlp = np.asarray(b_mlp, np.float32)
    has_bmlp = bool(np.any(b_mlp != 0.0))
    bmlp_col = np.ascontiguousarray(b_mlp[:, None]).astype(np.float32)  # [64, 1]

    bc_vec = (np.asarray(b_emb, np.float32) @ np.asarray(W_last, np.float32).T
              + np.asarray(b_last, np.float32))                  # [16]
    bc_t = np.tile(bc_vec[None, :], (128, 1)).astype(np.float32)

    x_pad = np.zeros((NP_PAD, FEAT), np.float32)
    x_pad[:N_NODES] = np.asarray(x, np.float32)
    xT = np.ascontiguousarray(x_pad.T.astype(bf))                # [256, NP_PAD] bf16

    wm2 = np.asarray(W_mlp, np.float32).T.reshape(2, 128, HID) \
        .transpose(1, 0, 2).reshape(128, 2 * HID).astype(bf)

    plan = dict(E_pad=E_pad, n_tiles=n_tiles, n_ech=n_ech, pairs=pairs,
                pair_of=pair_of, first_k=first_k, last_k=last_k,
                max_active=max_active, n_pairs=n_pairs,
                has_bnhh=has_bnhh, has_bmlp=has_bmlp, inv_S=inv_S,
                has_brz=bool(np.any(brz != 0.0)),
                has_bnih=bool(np.any(bnih != 0.0)),
                flushed=set(last_k.keys()), bc_vec=bc_vec)
    shared = dict(xT=xT, wm=np.ascontiguousarray(wm2), wih=wihS,
                  whh8=whh8, ba=ba_p,
                  brz=brz, bnih=bnih, bnhhS=bnhhS, bmlp=bmlp_col, bc=bc_t)
    percore = dict(gidx=gidx, oneh=oneh)
    return plan, shared, percore


def _build(plan):
    from contextlib import ExitStack
    import concourse.bass as bass  # noqa: F401
    import concourse.tile as tile
    from concourse import bacc, mybir

    f32 = mybir.dt.float32
    bf16 = mybir.dt.bfloat16
    f8 = mybir.dt.float8e4
    i16 = mybir.dt.int16
    AF = mybir.ActivationFunctionType
    OP = mybir.AluOpType
    DR = mybir.MatmulPerfMode.DoubleRow
    P = 128

    E_pad, n_tiles, n_ech = plan["E_pad"], plan["n_tiles"], plan["n_ech"]
    pairs, pair_of = plan["pairs"], plan["pair_of"]
    first_k, last_k = plan["first_k"], plan["last_k"]
    has_bnhh, has_bmlp = plan["has_bnhh"], plan["has_bmlp"]
    inv_S = plan["inv_S"]
    acc_bufs = min(6, max(2, plan["max_active"] + 1))

    nc = bacc.Bacc("TRN2", target_bir_lowering=False, debug=False)

    xT_d = nc.dram_tensor("xT", [FEAT, NP_PAD], bf16, kind="ExternalInput")
    wm_d = nc.dram_tensor("wm", [P, 2 * HID], bf16, kind="ExternalInput")
    wih_d = nc.dram_tensor("wih", [HID, G3], bf16, kind="ExternalInput")
    whh8_d = nc.dram_tensor("whh8", [P, 4 * G3], f8, kind="ExternalInput")
    ba_d = nc.dram_tensor("ba", [P, 4 * 136], bf16, kind="ExternalInput")
    brz_d = nc.dram_tensor("brz", [P, 8], f32, kind="ExternalInput")
    bnih_d = nc.dram_tensor("bnih", [P, 4], f32, kind="ExternalInput")
    bnhhS_d = nc.dram_tensor("bnhhS", [P, 4], f32, kind="ExternalInput")
    bmlp_d = nc.dram_tensor("bmlp", [HID, 1], f32, kind="ExternalInput")
    bc_d = nc.dram_tensor("bc", [P, OUT_DIM], f32, kind="ExternalInput")
    gidx_d = nc.dram_tensor("gidx", [n_tiles, P, (WALK * E_TILE) // 16], i16,
                            kind="ExternalInput")
    oneh_d = nc.dram_tensor("oneh", [max(plan["n_pairs"], 1), P, P], bf16,
                            kind="ExternalInput")
    out_d = nc.dram_tensor("out", [NODE_CHUNKS * P, OUT_DIM], f32,
                           kind="ExternalOutput")
    etab_d = nc.dram_tensor("etab", [NP_PAD, P], bf16, kind="Internal")

    from concourse.masks import make_identity

    with tile.TileContext(nc) as tc, ExitStack() as ctx:
        wpool = ctx.enter_context(tc.tile_pool(name="w", bufs=1))
        wih_sb = wpool.tile([HID, G3], bf16, tag="wih")
        nc.sync.dma_start(wih_sb[:], wih_d[:])
        whh8_sb = wpool.tile([P, 4, G3], f8, tag="whh8")
        nc.sync.dma_start(whh8_sb[:].rearrange("p a b -> p (a b)"), whh8_d[:])
        ba_sb = wpool.tile([P, 4 * 136], bf16, tag="ba")
        nc.sync.dma_start(ba_sb[:], ba_d[:])
        brz_sb = wpool.tile([P, 8], f32, tag="brz")
        nc.sync.dma_start(brz_sb[:], brz_d[:])
        bnih_sb = wpool.tile([P, 4], f32, tag="bnih")
        nc.sync.dma_start(bnih_sb[:], bnih_d[:])
        bnhhS_sb = wpool.tile([P, 4], f32, tag="bnhhS")
        nc.sync.dma_start(bnhhS_sb[:], bnhhS_d[:])
        bmlp_sb = wpool.tile([HID, 1], f32, tag="bmlp")
        nc.sync.dma_start(bmlp_sb[:], bmlp_d[:])
        bc_sb = wpool.tile([P, OUT_DIM], f32, tag="bc")
        nc.sync.dma_start(bc_sb[:], bc_d[:])
        wm_sb = wpool.tile([P, 2 * HID], bf16, tag="wm")
        nc.sync.dma_start(wm_sb[:], wm_d[:])
        ident_bf = wpool.tile([HID, HID], bf16, tag="ident")
        make_identity(nc, ident_bf[:])

        hpool = ctx.enter_context(tc.tile_pool(name="hT", bufs=1))
        hTf = hpool.tile([P, 4, E_pad], bf16, tag="hTf")

        # ---------------- phase 1: embedding table ----------------
        with tc.tile_pool(name="e_sb", bufs=3) as epool, \
             tc.tile_pool(name="e_ps", bufs=2, space="PSUM") as epsum:
            for it in range(NP_PAD // 512):
                r0 = it * 512
                xt = epool.tile([P, 2, 512], bf16, tag="xt")
                nc.sync.dma_start(
                    xt[:], xT_d[:, r0:r0 + 512].rearrange("(k p) n -> p k n", p=P))
                embp = epsum.tile([HID, 512], f32, tag="embp", space="PSUM")
                nc.tensor.matmul(embp[:], wm_sb[:, 0:HID], xt[:, 0, :],
                                 start=True, stop=False)
                nc.tensor.matmul(embp[:], wm_sb[:, HID:2 * HID], xt[:, 1, :],
                                 start=False, stop=True)
                embs = epool.tile([HID, 512], bf16, tag="embs")
                if has_bmlp:
                    nc.scalar.activation(embs[:], embp[:], AF.Identity,
                                         bias=bmlp_sb[:, 0:1])
                else:
                    nc.scalar.copy(embs[:], embp[:])
                pt = epsum.tile([P, 4, HID], bf16, tag="pt", space="PSUM")
                for j in range(4):
                    nc.tensor.matmul(pt[:, j, :], embs[:, j * P:(j + 1) * P],
                                     ident_bf[:], is_transpose=True,
                                     start=(j == 0), stop=(j == 3))
                esb = epool.tile([P, 4, P], bf16, tag="esb")
                nc.vector.memset(esb[:, :, HID:P], 0)
                nc.vector.tensor_copy(esb[:, :, 0:HID], pt[:])
                nc.sync.dma_start(
                    etab_d[r0:r0 + 512, :].rearrange("(j p) f -> p j f", p=P),
                    esb[:])

        # ---------------- phase 2: GRU over edge tiles ----------------
        NIDX = WALK * E_TILE
        with tc.tile_pool(name="g_idx", bufs=2) as ipool, \
             tc.tile_pool(name="g_gat", bufs=2) as gpool, \
             tc.tile_pool(name="g_rzn", bufs=2) as rznpool, \
             tc.tile_pool(name="g_h", bufs=2) as hspool, \
             tc.tile_pool(name="g_tmp", bufs=2) as tpool, \
             tc.tile_pool(name="g_ps", bufs=2, space="PSUM") as gpsum, \
             tc.tile_pool(name="g_px", bufs=1, space="PSUM") as gpsum_px, \
             tc.tile_pool(name="g_ph", bufs=1, space="PSUM") as gpsum_ph:

            def wih_s(m):
                return wih_sb[:, m * P:(m + 1) * P]

            zero_bias = not (plan["has_brz"] or plan["has_bnih"] or has_bnhh)

            for t in range(n_tiles):
                idxt = ipool.tile([P, NIDX // 16], i16, tag="idx")
                nc.sync.dma_start(idxt[:], gidx_d[t])
                gat = gpool.tile([P, 1, NIDX], bf16, tag="gat")
                nc.gpsimd.dma_gather(gat[:], etab_d[:], idxt[:], NIDX, NIDX, P,
                                     transpose=True, single_packet=False)

                def x_s(s):
                    return gat[0:HID, 0, s * E_TILE:(s + 1) * E_TILE]

                # ---- step 0 (h = 0): h1 = (1 - z)*n
                n_all = rznpool.tile([P, 4, E_TILE], bf16, tag="n")
                if zero_bias:
                    # sigmoid(-pre) = 1 - z directly; pair-batched activations
                    zn_all = rznpool.tile([P, 4, E_TILE], bf16, tag="z")
                    for pr in range(2):
                        c0 = pr * 2
                        gz = gpsum.tile([P, 2, E_TILE], f32, tag="g",
                                        space="PSUM")
                        for i in range(2):
                            nc.tensor.matmul(gz[:, i, :], wih_s(4 + c0 + i),
                                             x_s(0), start=True, stop=True)
                        nc.scalar.activation(zn_all[:, c0:c0 + 2, :], gz[:],
                                             AF.Sigmoid, scale=-inv_S)
                    for pr in range(2):
                        c0 = pr * 2
                        gn = gpsum.tile([P, 2, E_TILE], f32, tag="g",
                                        space="PSUM")
                        for i in range(2):
                            nc.tensor.matmul(gn[:, i, :], wih_s(8 + c0 + i),
                                             x_s(0), start=True, stop=True)
                        nc.scalar.activation(n_all[:, c0:c0 + 2, :], gn[:],
                                             AF.Tanh, scale=inv_S)
                    h_all = hspool.tile([P, 4, E_TILE], bf16, tag="h")
                    nc.vector.tensor_tensor(h_all[:], zn_all[:], n_all[:],
                                            OP.mult)
                else:
                    z_all = rznpool.tile([P, 4, E_TILE], bf16, tag="z")
                    for c in range(4):
                        pz = gpsum.tile([P, 2, E_TILE], f32, tag="g",
                                        space="PSUM")
                        nc.tensor.matmul(pz[:, 0, :], wih_s(4 + c), x_s(0),
                                         start=True, stop=True)
                        nc.scalar.activation(z_all[:, c, :], pz[:, 0, :],
                                             AF.Sigmoid,
                                             bias=brz_sb[:, 4 + c:5 + c],
                                             scale=inv_S)
                    r0_all = None
                    if has_bnhh:
                        r0_all = rznpool.tile([P, 4, E_TILE], bf16, tag="r")
                        for c in range(4):
                            prr = gpsum.tile([P, 2, E_TILE], f32, tag="g",
                                             space="PSUM")
                            nc.tensor.matmul(prr[:, 0, :], wih_s(c), x_s(0),
                                             start=True, stop=True)
                            nc.scalar.activation(r0_all[:, c, :], prr[:, 0, :],
                                                 AF.Sigmoid,
                                                 bias=brz_sb[:, c:c + 1],
                                                 scale=inv_S)
                    for c in range(4):
                        pn = gpsum.tile([P, 2, E_TILE], f32, tag="g",
                                        space="PSUM")
                        nc.tensor.matmul(pn[:, 0, :], wih_s(8 + c), x_s(0),
                                         start=True, stop=True)
                        if has_bnhh:
                            rb = tpool.tile([P, 2, E_TILE], f32, tag="rhn")
                            nc.vector.tensor_scalar(rb[:, 0, :], r0_all[:, c, :],
                                                    bnhhS_sb[:, c:c + 1], None,
                                                    OP.mult)
                            nc.vector.tensor_tensor(rb[:, 0, :], rb[:, 0, :],
                                                    pn[:, 0, :], OP.add)
                            nc.scalar.activation(n_all[:, c, :], rb[:, 0, :],
                                                 AF.Tanh,
                                                 bias=bnih_sb[:, c:c + 1],
                                                 scale=inv_S)
                        else:
                            nc.scalar.activation(n_all[:, c, :], pn[:, 0, :],
                                                 AF.Tanh,
                                                 bias=bnih_sb[:, c:c + 1],
                                                 scale=inv_S)
                    zn = tpool.tile([P, 4, E_TILE], bf16, tag="zn")
                    nc.vector.tensor_tensor(zn[:], z_all[:], n_all[:], OP.mult)
                    h_all = hspool.tile([P, 4, E_TILE], bf16, tag="h")
                    nc.vector.tensor_tensor(h_all[:], n_all[:], zn[:],
                                            OP.subtract)
                hq = hspool.tile([P, 4, E_TILE], f8, tag="hq")
                nc.gpsimd.tensor_scalar(hq[:], h_all[:], 128.0, None, OP.mult)

                # ---- steps 1..3
                for s in range(1, WALK):
                    final = (s == WALK - 1)
                    r_all = rznpool.tile([P, 4, E_TILE], bf16, tag="r")
                    z_all = rznpool.tile([P, 4, E_TILE], bf16, tag="z")
                    n_all = rznpool.tile([P, 4, E_TILE], bf16, tag="n")
                    for pr in range(4):
                        g2 = gpsum.tile([P, 2, E_TILE], f32, tag="g",
                                        space="PSUM")
                        for i in range(2):
                            m = pr * 2 + i
                            nc.tensor.matmul(g2[:, i, :], wih_s(m), x_s(s),
                                             start=True, stop=False)
                            nc.tensor.matmul(g2[:, i, :],
                                             whh8_sb[:, 0:2, m * P:(m + 1) * P],
                                             hq[:, 0:2, :], start=False,
                                             stop=False, perf_mode=DR,
                                             skip_group_check=True)
                            nc.tensor.matmul(g2[:, i, :],
                                             whh8_sb[:, 2:4, m * P:(m + 1) * P],
                                             hq[:, 2:4, :], start=False,
                                             stop=True, perf_mode=DR,
                                             skip_group_check=True)
                        if zero_bias:
                            dst_ap = (r_all[:, (pr % 2) * 2:(pr % 2) * 2 + 2, :]
                                      if pr < 2 else
                                      z_all[:, (pr % 2) * 2:(pr % 2) * 2 + 2, :])
                            nc.scalar.activation(dst_ap, g2[:], AF.Sigmoid,
                                                 scale=inv_S)
                        else:
                            for i in range(2):
                                m = pr * 2 + i
                                dst_ap = (r_all[:, m, :] if m < 4
                                          else z_all[:, m - 4, :])
                                nc.scalar.activation(dst_ap, g2[:, i, :],
                                                     AF.Sigmoid,
                                                     bias=brz_sb[:, m:m + 1],
                                                     scale=inv_S)
                    # n gates, pair-batched (c in {0,1} then {2,3})
                    for pr in range(2):
                        c0 = pr * 2
                        pxn = gpsum_px.tile([P, 2, E_TILE], f32, tag="px",
                                            space="PSUM")
                        phn = gpsum_ph.tile([P, 2, E_TILE], f32, tag="ph",
                                            space="PSUM")
                        for i in range(2):
                            m = 8 + c0 + i
                            nc.tensor.matmul(pxn[:, i, :], wih_s(m), x_s(s),
                                             start=True, stop=True)
                            nc.tensor.matmul(phn[:, i, :],
                                             whh8_sb[:, 0:2, m * P:(m + 1) * P],
                                             hq[:, 0:2, :], start=True, stop=False,
                                             perf_mode=DR, skip_group_check=True)
                            nc.tensor.matmul(phn[:, i, :],
                                             whh8_sb[:, 2:4, m * P:(m + 1) * P],
                                             hq[:, 2:4, :], start=False, stop=True,
                                             perf_mode=DR, skip_group_check=True)
                        if has_bnhh:
                            for i in range(2):
                                nc.vector.tensor_scalar(
                                    phn[:, i, :], phn[:, i, :],
                                    bnhhS_sb[:, c0 + i:c0 + i + 1], None, OP.add)
                        rhn = tpool.tile([P, 2, E_TILE], bf16, tag="rhn")
                        nc.vector.tensor_tensor(rhn[:], r_all[:, c0:c0 + 2, :],
                                                phn[:], OP.mult)
                        npre = tpool.tile([P, 2, E_TILE], bf16, tag="npre")
                        nc.vector.tensor_tensor(npre[:], rhn[:], pxn[:], OP.add)
                        if zero_bias:
                            nc.scalar.activation(n_all[:, c0:c0 + 2, :],
                                                 npre[:], AF.Tanh, scale=inv_S)
                        else:
                            for i in range(2):
                                nc.scalar.activation(
                                    n_all[:, c0 + i, :], npre[:, i, :], AF.Tanh,
                                    bias=bnih_sb[:, c0 + i:c0 + i + 1],
                                    scale=inv_S)
                    # state update (quad-batched)
                    h_prev = h_all
                    d4 = tpool.tile([P, 4, E_TILE], bf16, tag="d")
                    nc.vector.tensor_tensor(d4[:], h_prev[:], n_all[:],
                                            OP.subtract)
                    zd4 = tpool.tile([P, 4, E_TILE], bf16, tag="zd")
                    nc.vector.tensor_tensor(zd4[:], z_all[:], d4[:], OP.mult)
                    if final:
                        nc.vector.tensor_tensor(
                            hTf[:, :, t * E_TILE:(t + 1) * E_TILE],
                            n_all[:], zd4[:], OP.add)
                    else:
                        h_all = hspool.tile([P, 4, E_TILE], bf16, tag="h")
                        nc.vector.tensor_tensor(h_all[:], n_all[:], zd4[:],
                                                OP.add)
                        hq = hspool.tile([P, 4, E_TILE], f8, tag="hq")
                        nc.gpsimd.tensor_scalar(hq[:], h_all[:], 128.0, None,
                                                OP.mult)

        # ---------------- phase 3: attention + one-hot scatter ----------------
        with tc.tile_pool(name="s_sb", bufs=2) as spool, \
             tc.tile_pool(name="s_oh", bufs=4) as ohpool, \
             tc.tile_pool(name="s_pay", bufs=3) as paypool, \
             tc.tile_pool(name="s_ps", bufs=2, space="PSUM") as papsum, \
             tc.tile_pool(name="s_acc", bufs=acc_bufs, space="PSUM") as accpsum:

            chunk_pairs = {}
            for (k, j) in pairs:
                chunk_pairs.setdefault(k, []).append(j)
            acc = {}
            for k in range(n_ech):
                js = chunk_pairs.get(k)
                if not js:
                    continue
                pa = papsum.tile([P, 136], f32, tag="pa", space="PSUM")
                for kk in range(4):
                    nc.tensor.matmul(pa[:], hTf[:, kk, k * P:(k + 1) * P],
                                     ba_sb[:, kk * 136:(kk + 1) * 136],
                                     start=(kk == 0), stop=(kk == 3))
                asb = spool.tile([P, NH], f32, tag="asb")
                nc.vector.tensor_scalar(asb[:], pa[:, 128:136], 0.01, None,
                                        OP.mult)
                amx = spool.tile([P, NH], f32, tag="amx")
                nc.vector.tensor_tensor(amx[:], pa[:, 128:136], asb[:], OP.max)
                ea = spool.tile([P, NH], f32, tag="ea")
                nc.scalar.activation(ea[:], amx[:], AF.Exp)
                pay = paypool.tile([P, 136], bf16, tag="pay")
                nc.vector.tensor_tensor(
                    pay[:, 0:128].rearrange("p (h i) -> p h i", h=NH),
                    pa[:, 0:128].rearrange("p (h i) -> p h i", h=NH),
                    ea[:, :, None].to_broadcast([P, NH, OUT_DIM]), OP.mult)
                nc.scalar.copy(pay[:, 128:136], ea[:])
                for j in js:
                    pid = pair_of[(k, j)]
                    oh = ohpool.tile([P, P], bf16, tag="oh")
                    nc.sync.dma_start(oh[:], oneh_d[pid])
                    if first_k[j] == k:
                        acc[j] = accpsum.tile([P, 136], f32, tag="acc",
                                              name=f"acc{j}", space="PSUM")
                    nc.tensor.matmul(acc[j][:], oh[:], pay[:],
                                     start=(first_k[j] == k),
                                     stop=(last_k[j] == k),
                                     skip_group_check=True)
                for j in js:
                    if last_k[j] != k:
                        continue
                    aj = acc.pop(j)
                    sc = spool.tile([P, NH], f32, tag="sc")
                    nc.vector.tensor_scalar(sc[:], aj[:, 128:136], 1e-30, None,
                                            OP.max)
                    rc = spool.tile([P, NH], f32, tag="rc")
                    nc.vector.reciprocal(rc[:], sc[:])
                    wq = spool.tile([P, NH, OUT_DIM], f32, tag="wq")
                    nc.vector.tensor_tensor(
                        wq[:], aj[:, 0:128].rearrange("p (h i) -> p h i", h=NH),
                        rc[:, :, None].to_broadcast([P, NH, OUT_DIM]), OP.mult)
                    o16 = spool.tile([P, OUT_DIM], f32, tag="o16")
                    nc.vector.reduce_sum(
                        o16[:], wq[:].rearrange("p h i -> p i h"),
                        axis=mybir.AxisListType.X)
                    ob = spool.tile([P, OUT_DIM], f32, tag="ob")
                    nc.vector.tensor_tensor(ob[:], o16[:], bc_sb[:], OP.add)
                    nc.sync.dma_start(out_d[j * P:(j + 1) * P, :], ob[:])

    nc.compile()
    return nc


def kernel(**inputs):
    import os
    from concourse.bass_utils import run_bass_kernel_spmd

    num_nodes = int(inputs.pop("num_nodes", N_NODES))
    assert num_nodes == N_NODES
    plan, shared, percore = _host_prep(**inputs)
    nc = _build(plan)

    in_maps = []
    for c in range(N_CORES):
        m = dict(shared)
        m["gidx"] = np.ascontiguousarray(percore["gidx"][c])
        m["oneh"] = np.ascontiguousarray(percore["oneh"][c])
        in_maps.append(m)

    trace = bool(os.environ.get("KERNEL_TRACE"))
    res = run_bass_kernel_spmd(nc, in_maps, core_ids=list(range(N_CORES)),
                               trace=trace)
    global LAST_EXEC_NS, LAST_RESULTS
    LAST_EXEC_NS = getattr(res, "exec_time_ns", None)
    LAST_RESULTS = res

    full = np.empty((N_NODES, OUT_DIM), np.float32)
    for c in range(N_CORES):
        full[c * NPC:(c + 1) * NPC] = res.results[c]["out"][:NPC]
    # node chunks never flushed on device -> pure-bias rows
    for j in range(NODE_CHUNKS):
        if j not in plan["flushed"]:
            for c in range(N_CORES):
                lo = c * NPC + j * 128
                hi = min(c * NPC + min((j + 1) * 128, NPC), (c + 1) * NPC)
                if lo < hi:
                    full[lo:hi] = plan["bc_vec"][None, :]
    return full
